# revision 54
# baseline (speedup 1.0000x reference)
"""ParallelHyenaOperator Trainium2 kernel.

out = (irfft(rfft(u,2L) * rfft(k,2L))[:L] + u*d_bias) * x1,  u = x2*v, k = h*decay

Strategy: shard D=768 channels across 8 cores (96/core). Per channel, both
batches are packed into one complex FFT (z = u0 + i*u1); the conv theorem
gives y0 + i*y1 = ifft(fft(z) * fft(k)). The 16384-point FFT is a two-stage
radix-128 factorization on the tensor engine (bf16), with pointwise
twiddle/product stages on DVE/gpsimd and PSUM->SBUF evacuation split across
the scalar and vector engines.

Layout/instruction-count choices (v3):
- Host ships per-core arrays channel-major ([DPC, B*L] with (b, n2, q)
  flattened per channel), so every tensor loads/stores with ONE DMA per
  16-channel chunk (7 DMAs per chunk total).
- Batches are stacked along partitions ([p=(b n2), f=(c q)]); with
  re/im-stacked S1 weights [Wc_r; -Wc_i | Wc_i; Wc_r] each channel's S1 is
  a single N=256 matmul (pair-matmul: real and imag outputs side by side in
  one PSUM write). Same pairing applies to the inverse first stage (S1').
- The last inverse stage uses batch-stacked weights [W2c_r | W2c_i] so its
  output lands already (b n2)-stacked, and post-gating runs at full
  128-partition width directly from PSUM.
- x2/v/h/decay ship bf16 (they feed the bf16 FFT and the u product); x1
  ships f32 to protect the dominant (y + u*db)*x1 path. rel err ~5e-3.
"""

import math
import numpy as np
import ml_dtypes

B, D, L = 2, 768, 8192
NCORES = 8
DPC = D // NCORES          # channels per core = 96
NF = 2 * L                 # 16384 FFT size
C = 16                     # channels per chunk (and per cmul width)
NCHUNK = DPC // C          # 6
HG = 4                     # channels per matmul half-group
LOG_R_MIN, LOG_R_MAX = 0.0, 2.0

BF16 = ml_dtypes.bfloat16


def _make_consts():
    n2 = np.arange(64)
    n1 = np.arange(128)
    k1 = np.arange(128)
    k2 = np.arange(128)
    m2 = np.arange(64)

    Wc = np.exp(-2j * np.pi * np.outer(n2, k2) / 128)        # [64,128]
    T = np.exp(-2j * np.pi * np.outer(n1, k2) / NF)          # [128,128] [n1,k2]
    W2 = np.exp(-2j * np.pi * np.outer(n1, k1) / 128)        # [128,128]
    Wcc = np.exp(+2j * np.pi * np.outer(k1, n1) / 128)       # [128,128] [k1,m1]
    T2 = np.exp(+2j * np.pi * np.outer(k2, n1) / NF)         # [128,128] [k2,m1]
    W2c = np.exp(+2j * np.pi * np.outer(k2, m2) / 128) / NF  # [128,64]

    bf = lambda a: np.ascontiguousarray(a, dtype=np.float32).astype(BF16)
    c = {}
    # S1 stacked-complex pair weights [128, 256]: partition rows 0-63 act on
    # Re(z), 64-127 on Im(z); columns 0-127 produce Re, 128-255 produce Im.
    wst_r = np.concatenate([Wc.real, -Wc.imag], axis=0)      # [128,128]
    wst_i = np.concatenate([Wc.imag, Wc.real], axis=0)
    c["wstp"] = bf(np.concatenate([wst_r, wst_i], axis=1))   # [128,256]
    c["w2_r"] = bf(W2.real)
    c["w2_i"] = bf(W2.imag)
    c["w2_ni"] = bf(-W2.imag)
    # S1' pair weights [128, 256]
    c["wccp_a"] = bf(np.concatenate([Wcc.real, Wcc.imag], axis=1))
    c["wccp_b"] = bf(np.concatenate([-Wcc.imag, Wcc.real], axis=1))
    # twiddles ship once; replicated along channels via 0-stride APs
    c["t_r"] = bf(T.real)
    c["t_i"] = bf(T.imag)
    c["t2_r"] = bf(T2.real)
    c["t2_i"] = bf(T2.imag)
    # S2' batch-stacked weights [128, 128]
    c["w2cs_a"] = bf(np.concatenate([W2c.real, W2c.imag], axis=1))
    c["w2cs_b"] = bf(np.concatenate([-W2c.imag, W2c.real], axis=1))

    # decay = exp(-logspace(r)[d] * linspace(0,1,L)), module constant
    r = np.logspace(LOG_R_MIN, LOG_R_MAX, D).astype(np.float64)
    t = np.linspace(0.0, 1.0, L)
    decay = np.exp(-np.outer(r, t))
    c["_decay_full"] = np.ascontiguousarray(
        decay.astype(np.float32).astype(BF16))
    return c


_CONSTS = _make_consts()
_NC_CACHE = {}

CONST_NAMES = ["wstp", "w2_r", "w2_i", "w2_ni", "wccp_a", "wccp_b",
               "t_r", "t_i", "t2_r", "t2_i", "w2cs_a", "w2cs_b"]


def _build_nc():
    import concourse.bacc as bacc
    import concourse.tile as tile
    from concourse import mybir

    dt = mybir.dt
    AF = mybir.AluOpType

    nc = bacc.Bacc("TRN2", target_bir_lowering=False, debug=False,
                   num_devices=NCORES)

    def din(name, shape, d):
        return nc.dram_tensor(name, shape, d, kind="ExternalInput").ap()

    # channel-major layouts: [c, (b n2 q)] for B-tensors, [c, (n2 q)] else
    x1d = din("x1s", [DPC, B * L], dt.float32)
    x2d = din("x2s", [DPC, B * L], dt.bfloat16)
    vd = din("vs", [DPC, B * L], dt.bfloat16)
    hd = din("hs", [DPC, L], dt.bfloat16)
    decd = din("decays", [DPC, L], dt.bfloat16)
    dbd = din("db_rep", [1, DPC], dt.bfloat16)
    cc = {}
    for nm in CONST_NAMES:
        shp = list(_CONSTS[nm].shape)
        cc[nm] = din(nm, shp, dt.bfloat16)
    outd = nc.dram_tensor("out", [DPC, B * L], dt.float32,
                          kind="ExternalOutput").ap()

    CW = C * 128           # chunk free width (2048)
    HW = HG * 128          # half-group width (512)
    NHG = C // HG          # half-groups per chunk (4)

    with tile.TileContext(nc, trace_sim=False) as tc:
        cpool = tc.alloc_tile_pool(name="consts", bufs=1)
        iopool = tc.alloc_tile_pool(name="io", bufs=2)       # chunk inputs
        hdpool = tc.alloc_tile_pool(name="hd", bufs=1)       # h/dec chunk
        dbpool = tc.alloc_tile_pool(name="db", bufs=1)       # db broadcast
        upool = tc.alloc_tile_pool(name="u", bufs=2)         # z/k chunk tiles
        gpool = tc.alloc_tile_pool(name="grp", bufs=2)       # stage tiles
        tpool = tc.alloc_tile_pool(name="tmp", bufs=1)       # cmul temps
        opool = tc.alloc_tile_pool(name="out", bufs=2)       # post chunk tiles
        ps1 = tc.alloc_tile_pool(name="ps1", bufs=4, space="PSUM")
        ps2 = tc.alloc_tile_pool(name="ps2", bufs=2, space="PSUM")

        csb = {}
        for nm, ap in cc.items():
            t = cpool.tile(list(ap.shape), dt.bfloat16, tag=nm)
            nc.sync.dma_start(t[:], ap)
            csb[nm] = t

        def cmul(out_r, out_i, a_r, a_i, b_r, b_i, eng, tp, bcast=False):
            # (out_r + i*out_i) = (a_r + i*a_i) * (b_r + i*b_i), bf16.
            # All four products are materialized before either output is
            # written, so out_r/out_i may alias a_r/a_i. With bcast=True,
            # b_r/b_i are [128,128] constants replicated along channels via
            # a 0-stride AP.
            m1 = tpool.tile([128, CW], dt.bfloat16, tag=tp + "m1")
            m2 = tpool.tile([128, CW], dt.bfloat16, tag=tp + "m2")
            m3 = tpool.tile([128, CW], dt.bfloat16, tag=tp + "m3")
            m4 = tpool.tile([128, CW], dt.bfloat16, tag=tp + "m4")
            if bcast:
                r3 = lambda t: t[:].rearrange("p (c q) -> p c q", c=C)
                br = b_r[:].unsqueeze(1).broadcast_to([128, C, 128])
                bi = b_i[:].unsqueeze(1).broadcast_to([128, C, 128])
                eng.tensor_tensor(r3(m1), r3(a_r), br, AF.mult)
                eng.tensor_tensor(r3(m2), r3(a_i), bi, AF.mult)
                eng.tensor_tensor(r3(m3), r3(a_r), bi, AF.mult)
                eng.tensor_tensor(r3(m4), r3(a_i), br, AF.mult)
            else:
                eng.tensor_tensor(m1[:], a_r[:], b_r[:], AF.mult)
                eng.tensor_tensor(m2[:], a_i[:], b_i[:], AF.mult)
                eng.tensor_tensor(m3[:], a_r[:], b_i[:], AF.mult)
                eng.tensor_tensor(m4[:], a_i[:], b_r[:], AF.mult)
            eng.tensor_tensor(out_r[:], m1[:], m2[:], AF.subtract)
            eng.tensor_tensor(out_i[:], m3[:], m4[:], AF.add)

        def deinterleave(dst_r, dst_i, hsl, pair_ps, eng_r, eng_i):
            # pair_ps [128, HG*256] holds per-channel [re(128) | im(128)]
            src = pair_ps[:].rearrange("p (c q2) -> p c q2", q2=256)
            eng_r(dst_r[:, hsl].rearrange("p (c q) -> p c q", c=HG),
                  src[:, :, 0:128])
            eng_i(dst_i[:, hsl].rearrange("p (c q) -> p c q", c=HG),
                  src[:, :, 128:256])

        for ch in range(NCHUNK):
            c0 = ch * C
            # ---- chunk loads (one DMA per tensor) ----
            x2t = iopool.tile([128, CW], dt.bfloat16, tag="x2")
            vt = iopool.tile([128, CW], dt.bfloat16, tag="v")
            x1t = iopool.tile([128, CW], dt.float32, tag="x1")
            ht = hdpool.tile([64, CW], dt.bfloat16, tag="h")
            dct = hdpool.tile([64, CW], dt.bfloat16, tag="dec")
            dbt = dbpool.tile([128, C], dt.bfloat16, tag="db")
            for td, sd in ((x2t, x2d), (vt, vd), (x1t, x1d)):
                nc.sync.dma_start(
                    td[:].rearrange("k (c q) -> k c q", c=C),
                    sd[c0:c0 + C, :].rearrange("c (k q) -> k c q", k=128))
            nc.sync.dma_start(
                ht[:].rearrange("p (c q) -> p c q", c=C),
                hd[c0:c0 + C, :].rearrange("c (p q) -> p c q", p=64))
            nc.sync.dma_start(
                dct[:].rearrange("p (c q) -> p c q", c=C),
                decd[c0:c0 + C, :].rearrange("c (p q) -> p c q", p=64))
            nc.sync.dma_start(
                dbt[:], dbd[0:1, c0:c0 + C].broadcast_to([128, C]))

            # ---- chunk pointwise: z = x2*v, k = h*dec (bf16) ----
            zt = upool.tile([128, CW], dt.bfloat16, tag="z")
            nc.vector.tensor_tensor(zt[:], x2t[:], vt[:], AF.mult)
            kt = upool.tile([64, CW], dt.bfloat16, tag="k")
            nc.vector.tensor_tensor(kt[:], ht[:], dct[:], AF.mult)

            # stage tiles (chunk-wide bf16)
            z0rb = gpool.tile([128, CW], dt.bfloat16, tag="z0rb")
            z0ib = gpool.tile([128, CW], dt.bfloat16, tag="z0ib")
            zk0rb = gpool.tile([128, CW], dt.bfloat16, tag="zk0rb", bufs=1)
            zk0ib = gpool.tile([128, CW], dt.bfloat16, tag="zk0ib", bufs=1)

            # ---- S1: one pair-matmul per channel (u and k) ----
            for h2 in range(NHG):
                zu = ps2.tile([128, HG * 256], dt.float32, tag="ps2")
                zk = ps2.tile([128, HG * 256], dt.float32, tag="ps2")
                for j in range(HG):
                    csl = slice((h2 * HG + j) * 128, (h2 * HG + j + 1) * 128)
                    psl = slice(j * 256, (j + 1) * 256)
                    nc.tensor.matmul(zu[:, psl], zt[:, csl], csb["wstp"][:],
                                     start=True, stop=True)
                    nc.tensor.matmul(zk[:, psl], kt[:, csl],
                                     csb["wstp"][0:64, :],
                                     start=True, stop=True)
                hsl = slice(h2 * HW, (h2 + 1) * HW)
                deinterleave(z0rb, z0ib, hsl, zu,
                             nc.scalar.copy, nc.scalar.copy)
                deinterleave(zk0rb, zk0ib, hsl, zk,
                             nc.scalar.copy, nc.scalar.copy)

            # ---- forward twiddle (in place), k-side on gpsimd ----
            z1r, z1i, zk1r, zk1i = z0rb, z0ib, zk0rb, zk0ib
            cmul(z1r, z1i, z0rb, z0ib, csb["t_r"], csb["t_i"],
                 nc.vector, "v", bcast=True)
            cmul(zk1r, zk1i, zk0rb, zk0ib, csb["t_r"], csb["t_i"],
                 nc.gpsimd, "g", bcast=True)

            # ---- S2: grouped matmuls per half-group ----
            pzrb = gpool.tile([128, CW], dt.bfloat16, tag="pzrb")
            pzib = gpool.tile([128, CW], dt.bfloat16, tag="pzib")
            pkrb = gpool.tile([128, CW], dt.bfloat16, tag="pkrb")
            pkib = gpool.tile([128, CW], dt.bfloat16, tag="pkib")
            w2r, w2i, w2ni = csb["w2_r"], csb["w2_i"], csb["w2_ni"]
            for h2 in range(NHG):
                hsl = slice(h2 * HW, (h2 + 1) * HW)
                pzr = ps1.tile([128, HW], dt.float32, tag="ps1")
                pzi = ps1.tile([128, HW], dt.float32, tag="ps1")
                pkr = ps1.tile([128, HW], dt.float32, tag="ps1")
                pki = ps1.tile([128, HW], dt.float32, tag="ps1")
                nc.tensor.matmul(pzr[:], w2r[:], z1r[:, hsl], start=True, stop=False)
                nc.tensor.matmul(pzi[:], w2r[:], z1i[:, hsl], start=True, stop=False)
                nc.tensor.matmul(pkr[:], w2r[:], zk1r[:, hsl], start=True, stop=False)
                nc.tensor.matmul(pki[:], w2r[:], zk1i[:, hsl], start=True, stop=False)
                nc.tensor.matmul(pzi[:], w2i[:], z1r[:, hsl], start=False, stop=True)
                nc.tensor.matmul(pki[:], w2i[:], zk1r[:, hsl], start=False, stop=True)
                nc.tensor.matmul(pzr[:], w2ni[:], z1i[:, hsl], start=False, stop=True)
                nc.tensor.matmul(pkr[:], w2ni[:], zk1i[:, hsl], start=False, stop=True)
                nc.vector.tensor_copy(pzrb[:, hsl], pzr[:])
                nc.scalar.copy(pzib[:, hsl], pzi[:])
                nc.scalar.copy(pkrb[:, hsl], pkr[:])
                nc.scalar.copy(pkib[:, hsl], pki[:])

            # ---- spectral product (in place) ----
            pyr, pyi = pzrb, pzib
            cmul(pyr, pyi, pzrb, pzib, pkrb, pkib, nc.vector, "v")

            # ---- S1': two pair-matmuls per channel ----
            atrb = gpool.tile([128, CW], dt.bfloat16, tag="atrb")
            atib = gpool.tile([128, CW], dt.bfloat16, tag="atib")
            for h2 in range(NHG):
                atp = ps2.tile([128, HG * 256], dt.float32, tag="ps2")
                for j in range(HG):
                    csl = slice((h2 * HG + j) * 128, (h2 * HG + j + 1) * 128)
                    psl = slice(j * 256, (j + 1) * 256)
                    nc.tensor.matmul(atp[:, psl], pyr[:, csl],
                                     csb["wccp_a"][:], start=True, stop=False)
                    nc.tensor.matmul(atp[:, psl], pyi[:, csl],
                                     csb["wccp_b"][:], start=False, stop=True)
                hsl = slice(h2 * HW, (h2 + 1) * HW)
                deinterleave(atrb, atib, hsl, atp,
                             nc.scalar.copy, nc.vector.tensor_copy)

            # ---- inverse twiddle (in place, gpsimd) ----
            btr, bti = atrb, atib
            cmul(btr, bti, atrb, atib, csb["t2_r"], csb["t2_i"],
                 nc.gpsimd, "g", bcast=True)

            # ---- S2' (batch-stacked output) + post-gating from PSUM ----
            ott = opool.tile([128, CW], dt.float32, tag="ot")
            for h2 in range(NHG):
                hsl = slice(h2 * HW, (h2 + 1) * HW)
                yf = ps1.tile([128, HW], dt.float32, tag="ps1")
                nc.tensor.matmul(yf[:], csb["w2cs_a"][:], btr[:, hsl],
                                 start=True, stop=False)
                nc.tensor.matmul(yf[:], csb["w2cs_b"][:], bti[:, hsl],
                                 start=False, stop=True)
                x2db = tpool.tile([128, HW], dt.bfloat16, tag="x2db")
                dbr = (dbt[:, h2 * HG:(h2 + 1) * HG].unsqueeze(2)
                       .broadcast_to([128, HG, 128]))
                nc.gpsimd.tensor_tensor(
                    x2db[:].rearrange("p (c q) -> p c q", c=HG),
                    x2t[:, hsl].rearrange("p (c q) -> p c q", c=HG), dbr,
                    AF.mult)
                uf = tpool.tile([128, HW], dt.float32, tag="uf")
                nc.vector.tensor_tensor(uf[:], x2db[:], vt[:, hsl], AF.mult)
                nc.vector.tensor_tensor(uf[:], uf[:], yf[:], AF.add)
                nc.vector.tensor_tensor(ott[:, hsl], uf[:], x1t[:, hsl],
                                        AF.mult)

            # ---- out DMA (one per chunk) ----
            nc.sync.dma_start(
                outd[c0:c0 + C, :].rearrange("c (k q) -> k c q", k=128),
                ott[:].rearrange("k (c q) -> k c q", c=C))

        for p in (ps2, ps1, opool, tpool, gpool, upool, dbpool, hdpool,
                  iopool, cpool):
            p.release()

    nc.compile()
    return nc


def _get_nc():
    if "nc" not in _NC_CACHE:
        _NC_CACHE["nc"] = _build_nc()
    return _NC_CACHE["nc"]


def make_in_maps(x1, x2, v, h, d_bias):
    c = _CONSTS
    x1 = np.ascontiguousarray(x1, dtype=np.float32)
    x2bf = np.ascontiguousarray(x2, dtype=np.float32).astype(BF16)
    vbf = np.ascontiguousarray(v, dtype=np.float32).astype(BF16)
    hbf = np.ascontiguousarray(h, dtype=np.float32).astype(BF16)
    db = np.ascontiguousarray(d_bias, dtype=np.float32).astype(BF16)

    def cmajor(a):
        # [B, dpc, L] -> [dpc, B*L] with (b, n2, q) flattened per channel
        return np.ascontiguousarray(a.transpose(1, 0, 2).reshape(DPC, B * L))

    in_maps = []
    for core in range(NCORES):
        sl = slice(core * DPC, (core + 1) * DPC)
        m = {
            "x1s": cmajor(x1[:, sl]),
            "x2s": cmajor(x2bf[:, sl]),
            "vs": cmajor(vbf[:, sl]),
            "hs": np.ascontiguousarray(hbf[sl]),
            "decays": np.ascontiguousarray(c["_decay_full"][sl]),
            "db_rep": np.ascontiguousarray(db[sl][None, :]),
        }
        for nm in CONST_NAMES:
            m[nm] = c[nm]
        in_maps.append(m)
    return in_maps


def kernel(x1, x2, v, h, d_bias):
    from concourse import bass_utils

    nc = _get_nc()
    in_maps = make_in_maps(x1, x2, v, h, d_bias)
    res = bass_utils.run_bass_kernel_spmd(
        nc, in_maps, core_ids=list(range(NCORES)))
    # [dpc, B*L] per core -> [B, dpc, L], concat over cores
    outs = [r["out"].reshape(DPC, B, L).transpose(1, 0, 2)
            for r in res.results]
    out = np.concatenate(outs, axis=1)
    return np.ascontiguousarray(out.astype(np.float32))


if __name__ == "__main__":
    rng = np.random.default_rng(0)
    inputs = {
        "x1": rng.standard_normal((B, D, L)).astype(np.float32),
        "x2": rng.standard_normal((B, D, L)).astype(np.float32),
        "v": rng.standard_normal((B, D, L)).astype(np.float32),
        "h": (rng.standard_normal((D, L)) / math.sqrt(L) * 1e-5).astype(np.float32),
        "d_bias": rng.standard_normal(D).astype(np.float32),
    }
    out = kernel(**inputs)
    print(out.shape, out.dtype)


# revision 58
# speedup vs baseline: 1.0772x; 1.0772x over previous
"""ParallelHyenaOperator Trainium2 kernel.

out = (irfft(rfft(u,2L) * rfft(k,2L))[:L] + u*d_bias) * x1,  u = x2*v, k = h*decay

Strategy: shard D=768 channels across 8 cores (96/core). Per channel, both
batches are packed into one complex FFT (z = u0 + i*u1); the conv theorem
gives y0 + i*y1 = ifft(fft(z) * fft(k)). The 16384-point FFT is a two-stage
radix-128 factorization on the tensor engine (bf16), with pointwise
twiddle/product stages on DVE/gpsimd and PSUM->SBUF evacuation split across
the scalar and vector engines.

Layout/instruction-count choices (v3):
- Host ships per-core arrays channel-major ([DPC, B*L] with (b, n2, q)
  flattened per channel), so every tensor loads/stores with ONE DMA per
  16-channel chunk (7 DMAs per chunk total).
- Batches are stacked along partitions ([p=(b n2), f=(c q)]); with
  re/im-stacked S1 weights [Wc_r; -Wc_i | Wc_i; Wc_r] each channel's S1 is
  a single N=256 matmul (pair-matmul: real and imag outputs side by side in
  one PSUM write). Same pairing applies to the inverse first stage (S1').
- The last inverse stage uses batch-stacked weights [W2c_r | W2c_i] so its
  output lands already (b n2)-stacked, and post-gating runs at full
  128-partition width directly from PSUM.
- x2/v/h/decay ship bf16 (they feed the bf16 FFT and the u product); x1
  ships f32 to protect the dominant (y + u*db)*x1 path. rel err ~5e-3.
"""

import math
import numpy as np
import ml_dtypes

B, D, L = 2, 768, 8192
NCORES = 8
DPC = D // NCORES          # channels per core = 96
NF = 2 * L                 # 16384 FFT size
C = 16                     # channels per chunk (and per cmul width)
NCHUNK = DPC // C          # 6
HG = 4                     # channels per matmul half-group
LOG_R_MIN, LOG_R_MAX = 0.0, 2.0

BF16 = ml_dtypes.bfloat16


def _make_consts():
    n2 = np.arange(64)
    n1 = np.arange(128)
    k1 = np.arange(128)
    k2 = np.arange(128)
    m2 = np.arange(64)

    Wc = np.exp(-2j * np.pi * np.outer(n2, k2) / 128)        # [64,128]
    T = np.exp(-2j * np.pi * np.outer(n1, k2) / NF)          # [128,128] [n1,k2]
    W2 = np.exp(-2j * np.pi * np.outer(n1, k1) / 128)        # [128,128]
    Wcc = np.exp(+2j * np.pi * np.outer(k1, n1) / 128)       # [128,128] [k1,m1]
    T2 = np.exp(+2j * np.pi * np.outer(k2, n1) / NF)         # [128,128] [k2,m1]
    W2c = np.exp(+2j * np.pi * np.outer(k2, m2) / 128) / NF  # [128,64]

    bf = lambda a: np.ascontiguousarray(a, dtype=np.float32).astype(BF16)
    c = {}
    # S1 stacked-complex pair weights [128, 256]: partition rows 0-63 act on
    # Re(z), 64-127 on Im(z); columns 0-127 produce Re, 128-255 produce Im.
    wst_r = np.concatenate([Wc.real, -Wc.imag], axis=0)      # [128,128]
    wst_i = np.concatenate([Wc.imag, Wc.real], axis=0)
    c["wstp"] = bf(np.concatenate([wst_r, wst_i], axis=1))   # [128,256]
    c["w2_r"] = bf(W2.real)
    c["w2_i"] = bf(W2.imag)
    c["w2_ni"] = bf(-W2.imag)
    # S1' pair weights [128, 256]
    c["wccp_a"] = bf(np.concatenate([Wcc.real, Wcc.imag], axis=1))
    c["wccp_b"] = bf(np.concatenate([-Wcc.imag, Wcc.real], axis=1))
    # twiddles ship once; replicated along channels via 0-stride APs
    c["t_r"] = bf(T.real)
    c["t_i"] = bf(T.imag)
    c["t2_r"] = bf(T2.real)
    c["t2_i"] = bf(T2.imag)
    # S2' batch-stacked weights [128, 128]
    c["w2cs_a"] = bf(np.concatenate([W2c.real, W2c.imag], axis=1))
    c["w2cs_b"] = bf(np.concatenate([-W2c.imag, W2c.real], axis=1))

    # decay = exp(-logspace(r)[d] * linspace(0,1,L)), module constant
    r = np.logspace(LOG_R_MIN, LOG_R_MAX, D).astype(np.float64)
    t = np.linspace(0.0, 1.0, L)
    decay = np.exp(-np.outer(r, t))
    c["_decay_full"] = np.ascontiguousarray(
        decay.astype(np.float32).astype(BF16))
    return c


_CONSTS = _make_consts()
_NC_CACHE = {}

CONST_NAMES = ["wstp", "w2_r", "w2_i", "w2_ni", "wccp_a", "wccp_b",
               "t_r", "t_i", "t2_r", "t2_i", "w2cs_a", "w2cs_b"]


def _build_nc():
    import concourse.bacc as bacc
    import concourse.tile as tile
    from concourse import mybir

    dt = mybir.dt
    AF = mybir.AluOpType

    nc = bacc.Bacc("TRN2", target_bir_lowering=False, debug=False,
                   num_devices=NCORES)

    def din(name, shape, d):
        return nc.dram_tensor(name, shape, d, kind="ExternalInput").ap()

    # channel-major layouts: [c, (b n2 q)] for B-tensors, [c, (n2 q)] else
    x1d = din("x1s", [DPC, B * L], dt.float32)
    x2d = din("x2s", [DPC, B * L], dt.bfloat16)
    vd = din("vs", [DPC, B * L], dt.bfloat16)
    hd = din("hs", [DPC, L], dt.bfloat16)
    decd = din("decays", [DPC, L], dt.bfloat16)
    dbd = din("db_rep", [1, DPC], dt.bfloat16)
    cc = {}
    for nm in CONST_NAMES:
        shp = list(_CONSTS[nm].shape)
        cc[nm] = din(nm, shp, dt.bfloat16)
    outd = nc.dram_tensor("out", [DPC, B * L], dt.float32,
                          kind="ExternalOutput").ap()

    CW = C * 128           # chunk free width (2048)
    HW = HG * 128          # half-group width (512)
    NHG = C // HG          # half-groups per chunk (4)

    with tile.TileContext(nc, trace_sim=False) as tc:
        cpool = tc.alloc_tile_pool(name="consts", bufs=1)
        iopool = tc.alloc_tile_pool(name="io", bufs=2)       # chunk inputs
        hdpool = tc.alloc_tile_pool(name="hd", bufs=1)       # h/dec chunk
        dbpool = tc.alloc_tile_pool(name="db", bufs=1)       # db broadcast
        upool = tc.alloc_tile_pool(name="u", bufs=2)         # z/k chunk tiles
        gpool = tc.alloc_tile_pool(name="grp", bufs=2)       # stage tiles
        tpool = tc.alloc_tile_pool(name="tmp", bufs=1)       # cmul temps
        opool = tc.alloc_tile_pool(name="out", bufs=2)       # post chunk tiles
        psA = tc.alloc_tile_pool(name="psA", bufs=3, space="PSUM")
        psY = tc.alloc_tile_pool(name="psY", bufs=2, space="PSUM")

        csb = {}
        for nm, ap in cc.items():
            t = cpool.tile(list(ap.shape), dt.bfloat16, tag=nm)
            nc.sync.dma_start(t[:], ap)
            csb[nm] = t

        def cmul(out_r, out_i, a_r, a_i, b_r, b_i, eng, tp, bcast=False):
            # (out_r + i*out_i) = (a_r + i*a_i) * (b_r + i*b_i), bf16; all
            # operands are APs. All four products are materialized before
            # either output is written, so out_r/out_i may alias a_r/a_i.
            # With bcast=True, b_r/b_i are [128,128] constants replicated
            # along channels via a 0-stride AP.
            m1 = tpool.tile([128, CW], dt.bfloat16, tag=tp + "m1")
            m2 = tpool.tile([128, CW], dt.bfloat16, tag=tp + "m2")
            m3 = tpool.tile([128, CW], dt.bfloat16, tag=tp + "m3")
            m4 = tpool.tile([128, CW], dt.bfloat16, tag=tp + "m4")
            if bcast:
                r3 = lambda ap: ap.rearrange("p (c q) -> p c q", c=C)
                br = b_r.unsqueeze(1).broadcast_to([128, C, 128])
                bi = b_i.unsqueeze(1).broadcast_to([128, C, 128])
                eng.tensor_tensor(r3(m1[:]), r3(a_r), br, AF.mult)
                eng.tensor_tensor(r3(m2[:]), r3(a_i), bi, AF.mult)
                eng.tensor_tensor(r3(m3[:]), r3(a_r), bi, AF.mult)
                eng.tensor_tensor(r3(m4[:]), r3(a_i), br, AF.mult)
            else:
                eng.tensor_tensor(m1[:], a_r, b_r, AF.mult)
                eng.tensor_tensor(m2[:], a_i, b_i, AF.mult)
                eng.tensor_tensor(m3[:], a_r, b_i, AF.mult)
                eng.tensor_tensor(m4[:], a_i, b_r, AF.mult)
            eng.tensor_tensor(out_r, m1[:], m2[:], AF.subtract)
            eng.tensor_tensor(out_i, m3[:], m4[:], AF.add)

        def evac_pair(dst, hg, pair_ps, eng):
            # pair_ps [128, HG*256] holds per-channel [re(128) | im(128)];
            # dst [128, 2*CW] keeps re in [0:CW), im in [CW:2CW). One op.
            src = pair_ps[:].rearrange("p (c t q) -> p t c q", c=HG, t=2)
            d4 = (dst[:].rearrange("p (t c q) -> p t c q", t=2, c=C)
                  [:, :, hg * HG:(hg + 1) * HG, :])
            eng(d4, src)

        for ch in range(NCHUNK):
            c0 = ch * C
            # ---- chunk loads (one DMA per tensor) ----
            x2t = iopool.tile([128, CW], dt.bfloat16, tag="x2")
            vt = iopool.tile([128, CW], dt.bfloat16, tag="v")
            x1t = iopool.tile([128, CW], dt.float32, tag="x1")
            ht = hdpool.tile([64, CW], dt.bfloat16, tag="h")
            dct = hdpool.tile([64, CW], dt.bfloat16, tag="dec")
            dbt = dbpool.tile([128, C], dt.bfloat16, tag="db")
            for td, sd in ((x2t, x2d), (vt, vd), (x1t, x1d)):
                nc.sync.dma_start(
                    td[:].rearrange("k (c q) -> k c q", c=C),
                    sd[c0:c0 + C, :].rearrange("c (k q) -> k c q", k=128))
            nc.sync.dma_start(
                ht[:].rearrange("p (c q) -> p c q", c=C),
                hd[c0:c0 + C, :].rearrange("c (p q) -> p c q", p=64))
            nc.sync.dma_start(
                dct[:].rearrange("p (c q) -> p c q", c=C),
                decd[c0:c0 + C, :].rearrange("c (p q) -> p c q", p=64))
            nc.sync.dma_start(
                dbt[:], dbd[0:1, c0:c0 + C].broadcast_to([128, C]))

            # ---- chunk pointwise: z = x2*v, k = h*dec (bf16) ----
            zt = upool.tile([128, CW], dt.bfloat16, tag="z")
            nc.vector.tensor_tensor(zt[:], x2t[:], vt[:], AF.mult)
            kt = upool.tile([64, CW], dt.bfloat16, tag="k")
            nc.vector.tensor_tensor(kt[:], ht[:], dct[:], AF.mult)

            # merged stage tiles (re in [0:CW), im in [CW:2CW), bf16)
            zs = gpool.tile([128, 2 * CW], dt.bfloat16, tag="zs")
            zks = gpool.tile([128, 2 * CW], dt.bfloat16, tag="zks", bufs=1)

            # ---- S1: one pair-matmul per channel (u and k) ----
            for h2 in range(NHG):
                zu = psA.tile([128, HG * 256], dt.float32, tag="psA")
                zk = psA.tile([128, HG * 256], dt.float32, tag="psA")
                for j in range(HG):
                    csl = slice((h2 * HG + j) * 128, (h2 * HG + j + 1) * 128)
                    psl = slice(j * 256, (j + 1) * 256)
                    nc.tensor.matmul(zu[:, psl], zt[:, csl], csb["wstp"][:],
                                     start=True, stop=True)
                    nc.tensor.matmul(zk[:, psl], kt[:, csl],
                                     csb["wstp"][0:64, :],
                                     start=True, stop=True)
                evac_pair(zs, h2, zu, nc.scalar.copy)
                evac_pair(zks, h2, zk, nc.scalar.copy)

            # ---- forward twiddle (in place), k-side on gpsimd ----
            z1r, z1i = zs[:, 0:CW], zs[:, CW:2 * CW]
            zk1r, zk1i = zks[:, 0:CW], zks[:, CW:2 * CW]
            cmul(z1r, z1i, z1r, z1i, csb["t_r"][:], csb["t_i"][:],
                 nc.vector, "v", bcast=True)
            cmul(zk1r, zk1i, zk1r, zk1i, csb["t_r"][:], csb["t_i"][:],
                 nc.gpsimd, "g", bcast=True)

            # ---- S2: grouped matmuls per half-group ----
            pzs = gpool.tile([128, 2 * CW], dt.bfloat16, tag="pzs")
            pks = gpool.tile([128, 2 * CW], dt.bfloat16, tag="pks")
            w2r, w2i, w2ni = csb["w2_r"], csb["w2_i"], csb["w2_ni"]
            for h2 in range(NHG):
                hsl = slice(h2 * HW, (h2 + 1) * HW)
                pz2 = psA.tile([128, 2 * HW], dt.float32, tag="psA")
                pk2 = psA.tile([128, 2 * HW], dt.float32, tag="psA")
                pzr, pzi = pz2[:, 0:HW], pz2[:, HW:2 * HW]
                pkr, pki = pk2[:, 0:HW], pk2[:, HW:2 * HW]
                nc.tensor.matmul(pzr, w2r[:], z1r[:, hsl], start=True, stop=False)
                nc.tensor.matmul(pzi, w2r[:], z1i[:, hsl], start=True, stop=False)
                nc.tensor.matmul(pkr, w2r[:], zk1r[:, hsl], start=True, stop=False)
                nc.tensor.matmul(pki, w2r[:], zk1i[:, hsl], start=True, stop=False)
                nc.tensor.matmul(pzi, w2i[:], z1r[:, hsl], start=False, stop=True)
                nc.tensor.matmul(pki, w2i[:], zk1r[:, hsl], start=False, stop=True)
                nc.tensor.matmul(pzr, w2ni[:], z1i[:, hsl], start=False, stop=True)
                nc.tensor.matmul(pkr, w2ni[:], zk1i[:, hsl], start=False, stop=True)
                s2 = lambda t: t[:].rearrange("p (t2 x) -> p t2 x", t2=2)
                nc.vector.tensor_copy(s2(pzs)[:, :, hsl],
                                      pz2[:].rearrange("p (t2 x) -> p t2 x",
                                                       t2=2))
                nc.scalar.copy(s2(pks)[:, :, hsl],
                               pk2[:].rearrange("p (t2 x) -> p t2 x", t2=2))

            # ---- spectral product (in place) ----
            pyr, pyi = pzs[:, 0:CW], pzs[:, CW:2 * CW]
            cmul(pyr, pyi, pyr, pyi, pks[:, 0:CW], pks[:, CW:2 * CW],
                 nc.vector, "v")

            # ---- S1': two pair-matmuls per channel ----
            ats = gpool.tile([128, 2 * CW], dt.bfloat16, tag="ats")
            for h2 in range(NHG):
                atp = psA.tile([128, HG * 256], dt.float32, tag="psA")
                for j in range(HG):
                    csl = slice((h2 * HG + j) * 128, (h2 * HG + j + 1) * 128)
                    psl = slice(j * 256, (j + 1) * 256)
                    nc.tensor.matmul(atp[:, psl], pzs[:, csl],
                                     csb["wccp_a"][:], start=True, stop=False)
                    nc.tensor.matmul(atp[:, psl],
                                     pzs[:, CW + csl.start:CW + csl.stop],
                                     csb["wccp_b"][:], start=False, stop=True)
                evac_pair(ats, h2, atp, nc.scalar.copy)

            # ---- inverse twiddle (in place, gpsimd) ----
            btr, bti = ats[:, 0:CW], ats[:, CW:2 * CW]
            cmul(btr, bti, btr, bti, csb["t2_r"][:], csb["t2_i"][:],
                 nc.gpsimd, "g", bcast=True)

            # ---- S2' (batch-stacked output) + post-gating from PSUM ----
            ott = opool.tile([128, CW], dt.float32, tag="ot")
            for h2 in range(NHG):
                hsl = slice(h2 * HW, (h2 + 1) * HW)
                yf = psY.tile([128, HW], dt.float32, tag="psY")
                nc.tensor.matmul(yf[:], csb["w2cs_a"][:], btr[:, hsl],
                                 start=True, stop=False)
                nc.tensor.matmul(yf[:], csb["w2cs_b"][:], bti[:, hsl],
                                 start=False, stop=True)
                x2db = tpool.tile([128, HW], dt.bfloat16, tag="x2db")
                dbr = (dbt[:, h2 * HG:(h2 + 1) * HG].unsqueeze(2)
                       .broadcast_to([128, HG, 128]))
                nc.gpsimd.tensor_tensor(
                    x2db[:].rearrange("p (c q) -> p c q", c=HG),
                    x2t[:, hsl].rearrange("p (c q) -> p c q", c=HG), dbr,
                    AF.mult)
                uf = tpool.tile([128, HW], dt.float32, tag="uf")
                nc.vector.tensor_tensor(uf[:], x2db[:], vt[:, hsl], AF.mult)
                nc.vector.tensor_tensor(uf[:], uf[:], yf[:], AF.add)
                nc.vector.tensor_tensor(ott[:, hsl], uf[:], x1t[:, hsl],
                                        AF.mult)

            # ---- out DMA (one per chunk) ----
            nc.sync.dma_start(
                outd[c0:c0 + C, :].rearrange("c (k q) -> k c q", k=128),
                ott[:].rearrange("k (c q) -> k c q", c=C))

        for p in (psY, psA, opool, tpool, gpool, upool, dbpool, hdpool,
                  iopool, cpool):
            p.release()

    nc.compile()
    return nc


def _get_nc():
    if "nc" not in _NC_CACHE:
        _NC_CACHE["nc"] = _build_nc()
    return _NC_CACHE["nc"]


def make_in_maps(x1, x2, v, h, d_bias):
    c = _CONSTS
    x1 = np.ascontiguousarray(x1, dtype=np.float32)
    x2bf = np.ascontiguousarray(x2, dtype=np.float32).astype(BF16)
    vbf = np.ascontiguousarray(v, dtype=np.float32).astype(BF16)
    hbf = np.ascontiguousarray(h, dtype=np.float32).astype(BF16)
    db = np.ascontiguousarray(d_bias, dtype=np.float32).astype(BF16)

    def cmajor(a):
        # [B, dpc, L] -> [dpc, B*L] with (b, n2, q) flattened per channel
        return np.ascontiguousarray(a.transpose(1, 0, 2).reshape(DPC, B * L))

    in_maps = []
    for core in range(NCORES):
        sl = slice(core * DPC, (core + 1) * DPC)
        m = {
            "x1s": cmajor(x1[:, sl]),
            "x2s": cmajor(x2bf[:, sl]),
            "vs": cmajor(vbf[:, sl]),
            "hs": np.ascontiguousarray(hbf[sl]),
            "decays": np.ascontiguousarray(c["_decay_full"][sl]),
            "db_rep": np.ascontiguousarray(db[sl][None, :]),
        }
        for nm in CONST_NAMES:
            m[nm] = c[nm]
        in_maps.append(m)
    return in_maps


def kernel(x1, x2, v, h, d_bias):
    from concourse import bass_utils

    nc = _get_nc()
    in_maps = make_in_maps(x1, x2, v, h, d_bias)
    res = bass_utils.run_bass_kernel_spmd(
        nc, in_maps, core_ids=list(range(NCORES)))
    # [dpc, B*L] per core -> [B, dpc, L], concat over cores
    outs = [r["out"].reshape(DPC, B, L).transpose(1, 0, 2)
            for r in res.results]
    out = np.concatenate(outs, axis=1)
    return np.ascontiguousarray(out.astype(np.float32))


if __name__ == "__main__":
    rng = np.random.default_rng(0)
    inputs = {
        "x1": rng.standard_normal((B, D, L)).astype(np.float32),
        "x2": rng.standard_normal((B, D, L)).astype(np.float32),
        "v": rng.standard_normal((B, D, L)).astype(np.float32),
        "h": (rng.standard_normal((D, L)) / math.sqrt(L) * 1e-5).astype(np.float32),
        "d_bias": rng.standard_normal(D).astype(np.float32),
    }
    out = kernel(**inputs)
    print(out.shape, out.dtype)


# revision 62
# speedup vs baseline: 1.2310x; 1.1428x over previous
"""ParallelHyenaOperator Trainium2 kernel.

out = (irfft(rfft(u,2L) * rfft(k,2L))[:L] + u*d_bias) * x1,  u = x2*v, k = h*decay

Strategy: shard D=768 channels across 8 cores (96/core). Per channel, both
batches are packed into one complex FFT (z = u0 + i*u1); the conv theorem
gives y0 + i*y1 = ifft(fft(z) * fft(k)). The 16384-point FFT is a two-stage
radix-128 factorization on the tensor engine (bf16), with pointwise
twiddle/product stages on DVE/gpsimd and PSUM->SBUF evacuation split across
the scalar and vector engines.

Layout/instruction-count choices (v3):
- Host ships per-core arrays channel-major ([DPC, B*L] with (b, n2, q)
  flattened per channel), so every tensor loads/stores with ONE DMA per
  16-channel chunk (7 DMAs per chunk total).
- Batches are stacked along partitions ([p=(b n2), f=(c q)]); with
  re/im-stacked S1 weights [Wc_r; -Wc_i | Wc_i; Wc_r] each channel's S1 is
  a single N=256 matmul (pair-matmul: real and imag outputs side by side in
  one PSUM write). Same pairing applies to the inverse first stage (S1').
- The last inverse stage uses batch-stacked weights [W2c_r | W2c_i] so its
  output lands already (b n2)-stacked, and post-gating runs at full
  128-partition width directly from PSUM.
- x2/v/h/decay ship bf16 (they feed the bf16 FFT and the u product); x1
  ships f32 to protect the dominant (y + u*db)*x1 path. rel err ~5e-3.
"""

import math
import numpy as np
import ml_dtypes

B, D, L = 2, 768, 8192
NCORES = 8
DPC = D // NCORES          # channels per core = 96
NF = 2 * L                 # 16384 FFT size
C = 16                     # channels per chunk (and per cmul width)
NCHUNK = DPC // C          # 6
HG = 4                     # channels per matmul half-group
LOG_R_MIN, LOG_R_MAX = 0.0, 2.0

BF16 = ml_dtypes.bfloat16


def _make_consts():
    n2 = np.arange(64)
    n1 = np.arange(128)
    k1 = np.arange(128)
    k2 = np.arange(128)
    m2 = np.arange(64)

    Wc = np.exp(-2j * np.pi * np.outer(n2, k2) / 128)        # [64,128]
    T = np.exp(-2j * np.pi * np.outer(n1, k2) / NF)          # [128,128] [n1,k2]
    W2 = np.exp(-2j * np.pi * np.outer(n1, k1) / 128)        # [128,128]
    Wcc = np.exp(+2j * np.pi * np.outer(k1, n1) / 128)       # [128,128] [k1,m1]
    T2 = np.exp(+2j * np.pi * np.outer(k2, n1) / NF)         # [128,128] [k2,m1]
    W2c = np.exp(+2j * np.pi * np.outer(k2, m2) / 128) / NF  # [128,64]

    bf = lambda a: np.ascontiguousarray(a, dtype=np.float32).astype(BF16)
    c = {}
    # S1 stacked-complex pair weights [128, 256]: partition rows 0-63 act on
    # Re(z), 64-127 on Im(z); columns 0-127 produce Re, 128-255 produce Im.
    wst_r = np.concatenate([Wc.real, -Wc.imag], axis=0)      # [128,128]
    wst_i = np.concatenate([Wc.imag, Wc.real], axis=0)
    c["wstp"] = bf(np.concatenate([wst_r, wst_i], axis=1))   # [128,256]
    c["w2_r"] = bf(W2.real)
    c["w2_i"] = bf(W2.imag)
    c["w2_ni"] = bf(-W2.imag)
    # S1' pair weights [128, 256]
    c["wccp_a"] = bf(np.concatenate([Wcc.real, Wcc.imag], axis=1))
    c["wccp_b"] = bf(np.concatenate([-Wcc.imag, Wcc.real], axis=1))
    # twiddles ship once; replicated along channels via 0-stride APs
    c["t_r"] = bf(T.real)
    c["t_i"] = bf(T.imag)
    c["t2_r"] = bf(T2.real)
    c["t2_i"] = bf(T2.imag)
    # S2' batch-stacked weights [128, 128]
    c["w2cs_a"] = bf(np.concatenate([W2c.real, W2c.imag], axis=1))
    c["w2cs_b"] = bf(np.concatenate([-W2c.imag, W2c.real], axis=1))

    # decay = exp(-logspace(r)[d] * linspace(0,1,L)), module constant
    r = np.logspace(LOG_R_MIN, LOG_R_MAX, D).astype(np.float64)
    t = np.linspace(0.0, 1.0, L)
    decay = np.exp(-np.outer(r, t))
    c["_decay_full"] = np.ascontiguousarray(
        decay.astype(np.float32).astype(BF16))
    return c


_CONSTS = _make_consts()
_NC_CACHE = {}

CONST_NAMES = ["wstp", "w2_r", "w2_i", "w2_ni", "wccp_a", "wccp_b",
               "t_r", "t_i", "t2_r", "t2_i", "w2cs_a", "w2cs_b"]


def _build_nc():
    import concourse.bacc as bacc
    import concourse.tile as tile
    from concourse import mybir

    dt = mybir.dt
    AF = mybir.AluOpType

    nc = bacc.Bacc("TRN2", target_bir_lowering=False, debug=False,
                   num_devices=NCORES)

    def din(name, shape, d):
        return nc.dram_tensor(name, shape, d, kind="ExternalInput").ap()

    # channel-major layouts: [c, (b n2 q)] for B-tensors, [c, (n2 q)] else
    x1d = din("x1s", [DPC, B * L], dt.float32)
    x2d = din("x2s", [DPC, B * L], dt.bfloat16)
    vd = din("vs", [DPC, B * L], dt.bfloat16)
    hd = din("hs", [DPC, L], dt.bfloat16)
    decd = din("decays", [DPC, L], dt.bfloat16)
    dbd = din("db_rep", [1, DPC], dt.bfloat16)
    cc = {}
    for nm in CONST_NAMES:
        shp = list(_CONSTS[nm].shape)
        cc[nm] = din(nm, shp, dt.bfloat16)
    outd = nc.dram_tensor("out", [DPC, B * L], dt.float32,
                          kind="ExternalOutput").ap()

    CW = C * 128           # chunk free width (2048)
    HW = HG * 128          # half-group width (512)
    NHG = C // HG          # half-groups per chunk (4)

    with tile.TileContext(nc, trace_sim=False) as tc:
        cpool = tc.alloc_tile_pool(name="consts", bufs=1)
        iopool = tc.alloc_tile_pool(name="io", bufs=2)       # chunk inputs
        hdpool = tc.alloc_tile_pool(name="hd", bufs=1)       # h/dec chunk
        dbpool = tc.alloc_tile_pool(name="db", bufs=1)       # db broadcast
        upool = tc.alloc_tile_pool(name="u", bufs=2)         # z/k chunk tiles
        gpool = tc.alloc_tile_pool(name="grp", bufs=2)       # stage tiles
        tpool = tc.alloc_tile_pool(name="tmp", bufs=1)       # cmul temps
        opool = tc.alloc_tile_pool(name="out", bufs=2)       # post chunk tiles
        psA = tc.alloc_tile_pool(name="psA", bufs=3, space="PSUM")
        psY = tc.alloc_tile_pool(name="psY", bufs=2, space="PSUM")

        csb = {}
        for nm, ap in cc.items():
            t = cpool.tile(list(ap.shape), dt.bfloat16, tag=nm)
            nc.sync.dma_start(t[:], ap)
            csb[nm] = t

        def cmul(out_r, out_i, a_r, a_i, b_r, b_i, eng, tp, bcast=False):
            # (out_r + i*out_i) = (a_r + i*a_i) * (b_r + i*b_i), bf16; all
            # operands are APs. All four products are materialized before
            # either output is written, so out_r/out_i may alias a_r/a_i.
            # With bcast=True, b_r/b_i are [128,128] constants replicated
            # along channels via a 0-stride AP.
            m1 = tpool.tile([128, CW], dt.bfloat16, tag=tp + "m1")
            m2 = tpool.tile([128, CW], dt.bfloat16, tag=tp + "m2")
            m3 = tpool.tile([128, CW], dt.bfloat16, tag=tp + "m3")
            m4 = tpool.tile([128, CW], dt.bfloat16, tag=tp + "m4")
            if bcast:
                r3 = lambda ap: ap.rearrange("p (c q) -> p c q", c=C)
                br = b_r.unsqueeze(1).broadcast_to([128, C, 128])
                bi = b_i.unsqueeze(1).broadcast_to([128, C, 128])
                eng.tensor_tensor(r3(m1[:]), r3(a_r), br, AF.mult)
                eng.tensor_tensor(r3(m2[:]), r3(a_i), bi, AF.mult)
                eng.tensor_tensor(r3(m3[:]), r3(a_r), bi, AF.mult)
                eng.tensor_tensor(r3(m4[:]), r3(a_i), br, AF.mult)
            else:
                eng.tensor_tensor(m1[:], a_r, b_r, AF.mult)
                eng.tensor_tensor(m2[:], a_i, b_i, AF.mult)
                eng.tensor_tensor(m3[:], a_r, b_i, AF.mult)
                eng.tensor_tensor(m4[:], a_i, b_r, AF.mult)
            eng.tensor_tensor(out_r, m1[:], m2[:], AF.subtract)
            eng.tensor_tensor(out_i, m3[:], m4[:], AF.add)

        def evac_pair(dst, hg, pair_ps, eng):
            # pair_ps [128, HG*256] holds per-channel [re(128) | im(128)];
            # dst [128, 2*CW] keeps re in [0:CW), im in [CW:2CW). One op.
            src = pair_ps[:].rearrange("p (c t q) -> p t c q", c=HG, t=2)
            d4 = (dst[:].rearrange("p (t c q) -> p t c q", t=2, c=C)
                  [:, :, hg * HG:(hg + 1) * HG, :])
            eng(d4, src)

        for ch in range(NCHUNK):
            c0 = ch * C
            # ---- chunk loads (one DMA per tensor) ----
            x2t = iopool.tile([128, CW], dt.bfloat16, tag="x2")
            vt = iopool.tile([128, CW], dt.bfloat16, tag="v")
            x1t = iopool.tile([128, CW], dt.float32, tag="x1")
            ht = hdpool.tile([64, CW], dt.bfloat16, tag="h")
            dct = hdpool.tile([64, CW], dt.bfloat16, tag="dec")
            dbt = dbpool.tile([128, C], dt.bfloat16, tag="db")
            for td, sd in ((x2t, x2d), (vt, vd), (x1t, x1d)):
                nc.sync.dma_start(
                    td[:].rearrange("k (c q) -> k c q", c=C),
                    sd[c0:c0 + C, :].rearrange("c (k q) -> k c q", k=128))
            nc.sync.dma_start(
                ht[:].rearrange("p (c q) -> p c q", c=C),
                hd[c0:c0 + C, :].rearrange("c (p q) -> p c q", p=64))
            nc.sync.dma_start(
                dct[:].rearrange("p (c q) -> p c q", c=C),
                decd[c0:c0 + C, :].rearrange("c (p q) -> p c q", p=64))
            nc.sync.dma_start(
                dbt[:], dbd[0:1, c0:c0 + C].broadcast_to([128, C]))

            # ---- chunk pointwise: z = x2*v, k = h*dec (bf16, gpsimd) ----
            zt = upool.tile([128, CW], dt.bfloat16, tag="z")
            nc.gpsimd.tensor_tensor(zt[:], x2t[:], vt[:], AF.mult)
            kt = upool.tile([64, CW], dt.bfloat16, tag="k")
            nc.gpsimd.tensor_tensor(kt[:], ht[:], dct[:], AF.mult)

            # merged stage tiles (re in [0:CW), im in [CW:2CW), bf16)
            zs = gpool.tile([128, 2 * CW], dt.bfloat16, tag="zs")
            zks = gpool.tile([128, 2 * CW], dt.bfloat16, tag="zks", bufs=1)

            # ---- S1: one pair-matmul per channel (u and k) ----
            for h2 in range(NHG):
                zu = psA.tile([128, HG * 256], dt.float32, tag="psA")
                zk = psA.tile([128, HG * 256], dt.float32, tag="psA")
                for j in range(HG):
                    csl = slice((h2 * HG + j) * 128, (h2 * HG + j + 1) * 128)
                    psl = slice(j * 256, (j + 1) * 256)
                    nc.tensor.matmul(zu[:, psl], zt[:, csl], csb["wstp"][:],
                                     start=True, stop=True)
                    nc.tensor.matmul(zk[:, psl], kt[:, csl],
                                     csb["wstp"][0:64, :],
                                     start=True, stop=True)
                evac_pair(zs, h2, zu, nc.scalar.copy)
                evac_pair(zks, h2, zk, nc.scalar.copy)

            # ---- forward twiddle (in place), k-side on gpsimd ----
            z1r, z1i = zs[:, 0:CW], zs[:, CW:2 * CW]
            zk1r, zk1i = zks[:, 0:CW], zks[:, CW:2 * CW]
            cmul(z1r, z1i, z1r, z1i, csb["t_r"][:], csb["t_i"][:],
                 nc.vector, "v", bcast=True)
            cmul(zk1r, zk1i, zk1r, zk1i, csb["t_r"][:], csb["t_i"][:],
                 nc.gpsimd, "g", bcast=True)

            # ---- S2: grouped matmuls per half-group ----
            pzs = gpool.tile([128, 2 * CW], dt.bfloat16, tag="pzs")
            pks = gpool.tile([128, 2 * CW], dt.bfloat16, tag="pks")
            w2r, w2i, w2ni = csb["w2_r"], csb["w2_i"], csb["w2_ni"]
            for h2 in range(NHG):
                hsl = slice(h2 * HW, (h2 + 1) * HW)
                pz2 = psA.tile([128, 2 * HW], dt.float32, tag="psA")
                pk2 = psA.tile([128, 2 * HW], dt.float32, tag="psA")
                pzr, pzi = pz2[:, 0:HW], pz2[:, HW:2 * HW]
                pkr, pki = pk2[:, 0:HW], pk2[:, HW:2 * HW]
                nc.tensor.matmul(pzr, w2r[:], z1r[:, hsl], start=True, stop=False)
                nc.tensor.matmul(pzi, w2r[:], z1i[:, hsl], start=True, stop=False)
                nc.tensor.matmul(pkr, w2r[:], zk1r[:, hsl], start=True, stop=False)
                nc.tensor.matmul(pki, w2r[:], zk1i[:, hsl], start=True, stop=False)
                nc.tensor.matmul(pzi, w2i[:], z1r[:, hsl], start=False, stop=True)
                nc.tensor.matmul(pki, w2i[:], zk1r[:, hsl], start=False, stop=True)
                nc.tensor.matmul(pzr, w2ni[:], z1i[:, hsl], start=False, stop=True)
                nc.tensor.matmul(pkr, w2ni[:], zk1i[:, hsl], start=False, stop=True)
                s2 = lambda t: t[:].rearrange("p (t2 x) -> p t2 x", t2=2)
                nc.scalar.copy(s2(pzs)[:, :, hsl],
                               pz2[:].rearrange("p (t2 x) -> p t2 x", t2=2))
                nc.scalar.copy(s2(pks)[:, :, hsl],
                               pk2[:].rearrange("p (t2 x) -> p t2 x", t2=2))

            # ---- spectral product (in place) ----
            pyr, pyi = pzs[:, 0:CW], pzs[:, CW:2 * CW]
            cmul(pyr, pyi, pyr, pyi, pks[:, 0:CW], pks[:, CW:2 * CW],
                 nc.vector, "v")

            # ---- S1': two pair-matmuls per channel ----
            ats = gpool.tile([128, 2 * CW], dt.bfloat16, tag="ats")
            for h2 in range(NHG):
                atp = psA.tile([128, HG * 256], dt.float32, tag="psA")
                for j in range(HG):
                    csl = slice((h2 * HG + j) * 128, (h2 * HG + j + 1) * 128)
                    psl = slice(j * 256, (j + 1) * 256)
                    nc.tensor.matmul(atp[:, psl], pzs[:, csl],
                                     csb["wccp_a"][:], start=True, stop=False)
                    nc.tensor.matmul(atp[:, psl],
                                     pzs[:, CW + csl.start:CW + csl.stop],
                                     csb["wccp_b"][:], start=False, stop=True)
                evac_pair(ats, h2, atp, nc.scalar.copy)

            # ---- inverse twiddle (in place, gpsimd) ----
            btr, bti = ats[:, 0:CW], ats[:, CW:2 * CW]
            cmul(btr, bti, btr, bti, csb["t2_r"][:], csb["t2_i"][:],
                 nc.gpsimd, "g", bcast=True)

            # ---- S2' (batch-stacked output) + post-gating from PSUM ----
            ott = opool.tile([128, CW], dt.float32, tag="ot")
            for h2 in range(NHG):
                hsl = slice(h2 * HW, (h2 + 1) * HW)
                yf = psY.tile([128, HW], dt.float32, tag="psY")
                nc.tensor.matmul(yf[:], csb["w2cs_a"][:], btr[:, hsl],
                                 start=True, stop=False)
                nc.tensor.matmul(yf[:], csb["w2cs_b"][:], bti[:, hsl],
                                 start=False, stop=True)
                x2db = tpool.tile([128, HW], dt.bfloat16, tag="x2db")
                dbr = (dbt[:, h2 * HG:(h2 + 1) * HG].unsqueeze(2)
                       .broadcast_to([128, HG, 128]))
                nc.gpsimd.tensor_tensor(
                    x2db[:].rearrange("p (c q) -> p c q", c=HG),
                    x2t[:, hsl].rearrange("p (c q) -> p c q", c=HG), dbr,
                    AF.mult)
                uf = tpool.tile([128, HW], dt.float32, tag="uf")
                nc.vector.tensor_tensor(uf[:], x2db[:], vt[:, hsl], AF.mult)
                nc.vector.tensor_tensor(uf[:], uf[:], yf[:], AF.add)
                nc.gpsimd.tensor_tensor(ott[:, hsl], uf[:], x1t[:, hsl],
                                        AF.mult)

            # ---- out DMA (one per chunk) ----
            nc.sync.dma_start(
                outd[c0:c0 + C, :].rearrange("c (k q) -> k c q", k=128),
                ott[:].rearrange("k (c q) -> k c q", c=C))

        for p in (psY, psA, opool, tpool, gpool, upool, dbpool, hdpool,
                  iopool, cpool):
            p.release()

    nc.compile()
    return nc


def _get_nc():
    if "nc" not in _NC_CACHE:
        _NC_CACHE["nc"] = _build_nc()
    return _NC_CACHE["nc"]


def make_in_maps(x1, x2, v, h, d_bias):
    c = _CONSTS
    x1 = np.ascontiguousarray(x1, dtype=np.float32)
    x2bf = np.ascontiguousarray(x2, dtype=np.float32).astype(BF16)
    vbf = np.ascontiguousarray(v, dtype=np.float32).astype(BF16)
    hbf = np.ascontiguousarray(h, dtype=np.float32).astype(BF16)
    db = np.ascontiguousarray(d_bias, dtype=np.float32).astype(BF16)

    def cmajor(a):
        # [B, dpc, L] -> [dpc, B*L] with (b, n2, q) flattened per channel
        return np.ascontiguousarray(a.transpose(1, 0, 2).reshape(DPC, B * L))

    in_maps = []
    for core in range(NCORES):
        sl = slice(core * DPC, (core + 1) * DPC)
        m = {
            "x1s": cmajor(x1[:, sl]),
            "x2s": cmajor(x2bf[:, sl]),
            "vs": cmajor(vbf[:, sl]),
            "hs": np.ascontiguousarray(hbf[sl]),
            "decays": np.ascontiguousarray(c["_decay_full"][sl]),
            "db_rep": np.ascontiguousarray(db[sl][None, :]),
        }
        for nm in CONST_NAMES:
            m[nm] = c[nm]
        in_maps.append(m)
    return in_maps


def kernel(x1, x2, v, h, d_bias):
    from concourse import bass_utils

    nc = _get_nc()
    in_maps = make_in_maps(x1, x2, v, h, d_bias)
    res = bass_utils.run_bass_kernel_spmd(
        nc, in_maps, core_ids=list(range(NCORES)))
    # [dpc, B*L] per core -> [B, dpc, L], concat over cores
    outs = [r["out"].reshape(DPC, B, L).transpose(1, 0, 2)
            for r in res.results]
    out = np.concatenate(outs, axis=1)
    return np.ascontiguousarray(out.astype(np.float32))


if __name__ == "__main__":
    rng = np.random.default_rng(0)
    inputs = {
        "x1": rng.standard_normal((B, D, L)).astype(np.float32),
        "x2": rng.standard_normal((B, D, L)).astype(np.float32),
        "v": rng.standard_normal((B, D, L)).astype(np.float32),
        "h": (rng.standard_normal((D, L)) / math.sqrt(L) * 1e-5).astype(np.float32),
        "d_bias": rng.standard_normal(D).astype(np.float32),
    }
    out = kernel(**inputs)
    print(out.shape, out.dtype)


# revision 64
# speedup vs baseline: 1.7995x; 1.4618x over previous
"""ParallelHyenaOperator Trainium2 kernel.

out = (irfft(rfft(u,2L) * rfft(k,2L))[:L] + u*d_bias) * x1,  u = x2*v, k = h*decay

Strategy: shard D=768 channels across 8 cores (96/core). Per channel, both
batches are packed into one complex FFT (z = u0 + i*u1); the conv theorem
gives y0 + i*y1 = ifft(fft(z) * fft(k)). The 16384-point FFT is a two-stage
radix-128 factorization on the tensor engine (bf16), with pointwise
twiddle/product stages on DVE/gpsimd and PSUM->SBUF evacuation split across
the scalar and vector engines.

Layout/instruction-count choices (v3):
- Host ships per-core arrays channel-major ([DPC, B*L] with (b, n2, q)
  flattened per channel), so every tensor loads/stores with ONE DMA per
  16-channel chunk (7 DMAs per chunk total).
- Batches are stacked along partitions ([p=(b n2), f=(c q)]); with
  re/im-stacked S1 weights [Wc_r; -Wc_i | Wc_i; Wc_r] each channel's S1 is
  a single N=256 matmul (pair-matmul: real and imag outputs side by side in
  one PSUM write). Same pairing applies to the inverse first stage (S1').
- The last inverse stage uses batch-stacked weights [W2c_r | W2c_i] so its
  output lands already (b n2)-stacked, and post-gating runs at full
  128-partition width directly from PSUM.
- x2/v/h/decay ship bf16 (they feed the bf16 FFT and the u product); x1
  ships f32 to protect the dominant (y + u*db)*x1 path. rel err ~5e-3.
"""

import math
import numpy as np
import ml_dtypes

B, D, L = 2, 768, 8192
NCORES = 8
DPC = D // NCORES          # channels per core = 96
NF = 2 * L                 # 16384 FFT size
C = 16                     # channels per chunk (and per cmul width)
NCHUNK = DPC // C          # 6
HG = 4                     # channels per matmul half-group
LOG_R_MIN, LOG_R_MAX = 0.0, 2.0

BF16 = ml_dtypes.bfloat16


def _make_consts():
    n2 = np.arange(64)
    n1 = np.arange(128)
    k1 = np.arange(128)
    k2 = np.arange(128)
    m2 = np.arange(64)

    Wc = np.exp(-2j * np.pi * np.outer(n2, k2) / 128)        # [64,128]
    T = np.exp(-2j * np.pi * np.outer(n1, k2) / NF)          # [128,128] [n1,k2]
    W2 = np.exp(-2j * np.pi * np.outer(n1, k1) / 128)        # [128,128]
    Wcc = np.exp(+2j * np.pi * np.outer(k1, n1) / 128)       # [128,128] [k1,m1]
    T2 = np.exp(+2j * np.pi * np.outer(k2, n1) / NF)         # [128,128] [k2,m1]
    W2c = np.exp(+2j * np.pi * np.outer(k2, m2) / 128) / NF  # [128,64]

    bf = lambda a: np.ascontiguousarray(a, dtype=np.float32).astype(BF16)
    c = {}
    # S1 stacked-complex pair weights [128, 256]: partition rows 0-63 act on
    # Re(z), 64-127 on Im(z); columns 0-127 produce Re, 128-255 produce Im.
    wst_r = np.concatenate([Wc.real, -Wc.imag], axis=0)      # [128,128]
    wst_i = np.concatenate([Wc.imag, Wc.real], axis=0)
    c["wstp"] = bf(np.concatenate([wst_r, wst_i], axis=1))   # [128,256]
    c["w2_r"] = bf(W2.real)
    c["w2_i"] = bf(W2.imag)
    c["w2_ni"] = bf(-W2.imag)
    # S1' pair weights [128, 256]
    c["wccp_a"] = bf(np.concatenate([Wcc.real, Wcc.imag], axis=1))
    c["wccp_b"] = bf(np.concatenate([-Wcc.imag, Wcc.real], axis=1))
    # twiddles ship once; replicated along channels via 0-stride APs
    c["t_r"] = bf(T.real)
    c["t_i"] = bf(T.imag)
    c["t2_r"] = bf(T2.real)
    c["t2_i"] = bf(T2.imag)
    # S2' batch-stacked weights [128, 128]
    c["w2cs_a"] = bf(np.concatenate([W2c.real, W2c.imag], axis=1))
    c["w2cs_b"] = bf(np.concatenate([-W2c.imag, W2c.real], axis=1))

    # decay = exp(-logspace(r)[d] * linspace(0,1,L)), module constant
    r = np.logspace(LOG_R_MIN, LOG_R_MAX, D).astype(np.float64)
    t = np.linspace(0.0, 1.0, L)
    decay = np.exp(-np.outer(r, t))
    c["_decay_full"] = np.ascontiguousarray(
        decay.astype(np.float32).astype(BF16))
    return c


_CONSTS = _make_consts()
_NC_CACHE = {}

CONST_NAMES = ["wstp", "w2_r", "w2_i", "w2_ni", "wccp_a", "wccp_b",
               "t_r", "t_i", "t2_r", "t2_i", "w2cs_a", "w2cs_b"]


def _build_nc():
    import concourse.bacc as bacc
    import concourse.tile as tile
    from concourse import mybir

    dt = mybir.dt
    AF = mybir.AluOpType

    nc = bacc.Bacc("TRN2", target_bir_lowering=False, debug=False,
                   num_devices=NCORES)

    def din(name, shape, d):
        return nc.dram_tensor(name, shape, d, kind="ExternalInput").ap()

    # channel-major layouts: [c, (b n2 q)] for B-tensors, [c, (n2 q)] else
    x1d = din("x1s", [DPC, B * L], dt.float32)
    x2d = din("x2s", [DPC, B * L], dt.bfloat16)
    vd = din("vs", [DPC, B * L], dt.bfloat16)
    hd = din("hs", [DPC, L], dt.bfloat16)
    decd = din("decays", [DPC, L], dt.bfloat16)
    dbd = din("db_rep", [1, DPC], dt.bfloat16)
    cc = {}
    for nm in CONST_NAMES:
        shp = list(_CONSTS[nm].shape)
        cc[nm] = din(nm, shp, dt.bfloat16)
    outd = nc.dram_tensor("out", [DPC, B * L], dt.float32,
                          kind="ExternalOutput").ap()

    CW = C * 128           # chunk free width (2048)
    HW = HG * 128          # half-group width (512)
    NHG = C // HG          # half-groups per chunk (4)

    with tile.TileContext(nc, trace_sim=False) as tc:
        cpool = tc.alloc_tile_pool(name="consts", bufs=1)
        iopool = tc.alloc_tile_pool(name="io", bufs=2)       # chunk inputs
        hdpool = tc.alloc_tile_pool(name="hd", bufs=1)       # h/dec chunk
        dbpool = tc.alloc_tile_pool(name="db", bufs=1)       # db broadcast
        upool = tc.alloc_tile_pool(name="u", bufs=2)         # z/k chunk tiles
        gpool = tc.alloc_tile_pool(name="grp", bufs=2)       # stage tiles
        tpool = tc.alloc_tile_pool(name="tmp", bufs=1)       # cmul temps
        opool = tc.alloc_tile_pool(name="out", bufs=2)       # post chunk tiles
        psA = tc.alloc_tile_pool(name="psA", bufs=3, space="PSUM")
        psY = tc.alloc_tile_pool(name="psY", bufs=2, space="PSUM")

        csb = {}
        for nm, ap in cc.items():
            t = cpool.tile(list(ap.shape), dt.bfloat16, tag=nm)
            nc.sync.dma_start(t[:], ap)
            csb[nm] = t

        def cmul(out_r, out_i, a_r, a_i, b_r, b_i, eng, tp, bcast=False):
            # (out_r + i*out_i) = (a_r + i*a_i) * (b_r + i*b_i), bf16; all
            # operands are APs. All four products are materialized before
            # either output is written, so out_r/out_i may alias a_r/a_i.
            # With bcast=True, b_r/b_i are [128,128] constants replicated
            # along channels via a 0-stride AP.
            m1 = tpool.tile([128, CW], dt.bfloat16, tag=tp + "m1")
            m2 = tpool.tile([128, CW], dt.bfloat16, tag=tp + "m2")
            m3 = tpool.tile([128, CW], dt.bfloat16, tag=tp + "m3")
            m4 = tpool.tile([128, CW], dt.bfloat16, tag=tp + "m4")
            if bcast:
                r3 = lambda ap: ap.rearrange("p (c q) -> p c q", c=C)
                br = b_r.unsqueeze(1).broadcast_to([128, C, 128])
                bi = b_i.unsqueeze(1).broadcast_to([128, C, 128])
                eng.tensor_tensor(r3(m1[:]), r3(a_r), br, AF.mult)
                eng.tensor_tensor(r3(m2[:]), r3(a_i), bi, AF.mult)
                eng.tensor_tensor(r3(m3[:]), r3(a_r), bi, AF.mult)
                eng.tensor_tensor(r3(m4[:]), r3(a_i), br, AF.mult)
            else:
                eng.tensor_tensor(m1[:], a_r, b_r, AF.mult)
                eng.tensor_tensor(m2[:], a_i, b_i, AF.mult)
                eng.tensor_tensor(m3[:], a_r, b_i, AF.mult)
                eng.tensor_tensor(m4[:], a_i, b_r, AF.mult)
            eng.tensor_tensor(out_r, m1[:], m2[:], AF.subtract)
            eng.tensor_tensor(out_i, m3[:], m4[:], AF.add)

        def evac_pair(dst, hg, pair_ps, eng):
            # pair_ps [128, HG*256] holds per-channel [re(128) | im(128)];
            # dst [128, 2*CW] keeps re in [0:CW), im in [CW:2CW). One op.
            src = pair_ps[:].rearrange("p (c t q) -> p t c q", c=HG, t=2)
            d4 = (dst[:].rearrange("p (t c q) -> p t c q", t=2, c=C)
                  [:, :, hg * HG:(hg + 1) * HG, :])
            eng(d4, src)

        for ch in range(NCHUNK):
            c0 = ch * C
            # ---- chunk loads (one DMA per tensor) ----
            x2t = iopool.tile([128, CW], dt.bfloat16, tag="x2")
            vt = iopool.tile([128, CW], dt.bfloat16, tag="v")
            x1t = iopool.tile([128, CW], dt.float32, tag="x1")
            ht = hdpool.tile([64, CW], dt.bfloat16, tag="h")
            dct = hdpool.tile([64, CW], dt.bfloat16, tag="dec")
            dbt = dbpool.tile([128, C], dt.bfloat16, tag="db")
            for td, sd in ((x2t, x2d), (vt, vd), (x1t, x1d)):
                nc.sync.dma_start(
                    td[:].rearrange("k (c q) -> k c q", c=C),
                    sd[c0:c0 + C, :].rearrange("c (k q) -> k c q", k=128))
            nc.sync.dma_start(
                ht[:].rearrange("p (c q) -> p c q", c=C),
                hd[c0:c0 + C, :].rearrange("c (p q) -> p c q", p=64))
            nc.sync.dma_start(
                dct[:].rearrange("p (c q) -> p c q", c=C),
                decd[c0:c0 + C, :].rearrange("c (p q) -> p c q", p=64))
            nc.sync.dma_start(
                dbt[:], dbd[0:1, c0:c0 + C].broadcast_to([128, C]))

            # ---- chunk pointwise: z = x2*v, k = h*dec (bf16) ----
            zt = upool.tile([128, CW], dt.bfloat16, tag="z")
            nc.vector.tensor_tensor(zt[:], x2t[:], vt[:], AF.mult)
            kt = upool.tile([64, CW], dt.bfloat16, tag="k")
            nc.vector.tensor_tensor(kt[:], ht[:], dct[:], AF.mult)

            # merged stage tiles (re in [0:CW), im in [CW:2CW), bf16)
            zs = gpool.tile([128, 2 * CW], dt.bfloat16, tag="zs")
            zks = gpool.tile([128, 2 * CW], dt.bfloat16, tag="zks", bufs=1)

            # ---- S1: one pair-matmul per channel (u and k) ----
            for h2 in range(NHG):
                zu = psA.tile([128, HG * 256], dt.float32, tag="psA")
                zk = psA.tile([128, HG * 256], dt.float32, tag="psA")
                for j in range(HG):
                    csl = slice((h2 * HG + j) * 128, (h2 * HG + j + 1) * 128)
                    psl = slice(j * 256, (j + 1) * 256)
                    nc.tensor.matmul(zu[:, psl], zt[:, csl], csb["wstp"][:],
                                     start=True, stop=True)
                    nc.tensor.matmul(zk[:, psl], kt[:, csl],
                                     csb["wstp"][0:64, :],
                                     start=True, stop=True)
                evac_pair(zs, h2, zu, nc.scalar.copy)
                evac_pair(zks, h2, zk, nc.scalar.copy)

            # ---- forward twiddle (in place), k-side on gpsimd ----
            z1r, z1i = zs[:, 0:CW], zs[:, CW:2 * CW]
            zk1r, zk1i = zks[:, 0:CW], zks[:, CW:2 * CW]
            cmul(z1r, z1i, z1r, z1i, csb["t_r"][:], csb["t_i"][:],
                 nc.vector, "v", bcast=True)
            cmul(zk1r, zk1i, zk1r, zk1i, csb["t_r"][:], csb["t_i"][:],
                 nc.gpsimd, "g", bcast=True)

            # ---- S2: grouped matmuls per half-group ----
            pzs = gpool.tile([128, 2 * CW], dt.bfloat16, tag="pzs")
            pks = gpool.tile([128, 2 * CW], dt.bfloat16, tag="pks")
            w2r, w2i, w2ni = csb["w2_r"], csb["w2_i"], csb["w2_ni"]
            for h2 in range(NHG):
                hsl = slice(h2 * HW, (h2 + 1) * HW)
                pz2 = psA.tile([128, 2 * HW], dt.float32, tag="psA")
                pk2 = psA.tile([128, 2 * HW], dt.float32, tag="psA")
                pzr, pzi = pz2[:, 0:HW], pz2[:, HW:2 * HW]
                pkr, pki = pk2[:, 0:HW], pk2[:, HW:2 * HW]
                nc.tensor.matmul(pzr, w2r[:], z1r[:, hsl], start=True, stop=False)
                nc.tensor.matmul(pzi, w2r[:], z1i[:, hsl], start=True, stop=False)
                nc.tensor.matmul(pkr, w2r[:], zk1r[:, hsl], start=True, stop=False)
                nc.tensor.matmul(pki, w2r[:], zk1i[:, hsl], start=True, stop=False)
                nc.tensor.matmul(pzi, w2i[:], z1r[:, hsl], start=False, stop=True)
                nc.tensor.matmul(pki, w2i[:], zk1r[:, hsl], start=False, stop=True)
                nc.tensor.matmul(pzr, w2ni[:], z1i[:, hsl], start=False, stop=True)
                nc.tensor.matmul(pkr, w2ni[:], zk1i[:, hsl], start=False, stop=True)
                s2 = lambda t: t[:].rearrange("p (t2 x) -> p t2 x", t2=2)
                nc.scalar.copy(s2(pzs)[:, :, hsl],
                               pz2[:].rearrange("p (t2 x) -> p t2 x", t2=2))
                nc.scalar.copy(s2(pks)[:, :, hsl],
                               pk2[:].rearrange("p (t2 x) -> p t2 x", t2=2))

            # ---- spectral product (in place) ----
            pyr, pyi = pzs[:, 0:CW], pzs[:, CW:2 * CW]
            cmul(pyr, pyi, pyr, pyi, pks[:, 0:CW], pks[:, CW:2 * CW],
                 nc.vector, "v")

            # ---- S1': two pair-matmuls per channel ----
            ats = gpool.tile([128, 2 * CW], dt.bfloat16, tag="ats")
            for h2 in range(NHG):
                atp = psA.tile([128, HG * 256], dt.float32, tag="psA")
                for j in range(HG):
                    csl = slice((h2 * HG + j) * 128, (h2 * HG + j + 1) * 128)
                    psl = slice(j * 256, (j + 1) * 256)
                    nc.tensor.matmul(atp[:, psl], pzs[:, csl],
                                     csb["wccp_a"][:], start=True, stop=False)
                    nc.tensor.matmul(atp[:, psl],
                                     pzs[:, CW + csl.start:CW + csl.stop],
                                     csb["wccp_b"][:], start=False, stop=True)
                evac_pair(ats, h2, atp, nc.scalar.copy)

            # ---- inverse twiddle (in place, gpsimd) ----
            btr, bti = ats[:, 0:CW], ats[:, CW:2 * CW]
            cmul(btr, bti, btr, bti, csb["t2_r"][:], csb["t2_i"][:],
                 nc.gpsimd, "g", bcast=True)

            # ---- S2' (batch-stacked output) + post-gating from PSUM ----
            ott = opool.tile([128, CW], dt.float32, tag="ot")
            for h2 in range(NHG):
                hsl = slice(h2 * HW, (h2 + 1) * HW)
                yf = psY.tile([128, HW], dt.float32, tag="psY")
                nc.tensor.matmul(yf[:], csb["w2cs_a"][:], btr[:, hsl],
                                 start=True, stop=False)
                nc.tensor.matmul(yf[:], csb["w2cs_b"][:], bti[:, hsl],
                                 start=False, stop=True)
                x2db = tpool.tile([128, HW], dt.bfloat16, tag="x2db")
                dbr = (dbt[:, h2 * HG:(h2 + 1) * HG].unsqueeze(2)
                       .broadcast_to([128, HG, 128]))
                nc.gpsimd.tensor_tensor(
                    x2db[:].rearrange("p (c q) -> p c q", c=HG),
                    x2t[:, hsl].rearrange("p (c q) -> p c q", c=HG), dbr,
                    AF.mult)
                uf = tpool.tile([128, HW], dt.float32, tag="uf")
                nc.vector.tensor_tensor(uf[:], x2db[:], vt[:, hsl], AF.mult)
                nc.vector.tensor_tensor(uf[:], uf[:], yf[:], AF.add)
                nc.vector.tensor_tensor(ott[:, hsl], uf[:], x1t[:, hsl],
                                        AF.mult)

            # ---- out DMA (one per chunk) ----
            nc.sync.dma_start(
                outd[c0:c0 + C, :].rearrange("c (k q) -> k c q", k=128),
                ott[:].rearrange("k (c q) -> k c q", c=C))

        for p in (psY, psA, opool, tpool, gpool, upool, dbpool, hdpool,
                  iopool, cpool):
            p.release()

    nc.compile()
    return nc


def _get_nc():
    if "nc" not in _NC_CACHE:
        _NC_CACHE["nc"] = _build_nc()
    return _NC_CACHE["nc"]


def make_in_maps(x1, x2, v, h, d_bias):
    c = _CONSTS
    x1 = np.ascontiguousarray(x1, dtype=np.float32)
    x2bf = np.ascontiguousarray(x2, dtype=np.float32).astype(BF16)
    vbf = np.ascontiguousarray(v, dtype=np.float32).astype(BF16)
    hbf = np.ascontiguousarray(h, dtype=np.float32).astype(BF16)
    db = np.ascontiguousarray(d_bias, dtype=np.float32).astype(BF16)

    def cmajor(a):
        # [B, dpc, L] -> [dpc, B*L] with (b, n2, q) flattened per channel
        return np.ascontiguousarray(a.transpose(1, 0, 2).reshape(DPC, B * L))

    in_maps = []
    for core in range(NCORES):
        sl = slice(core * DPC, (core + 1) * DPC)
        m = {
            "x1s": cmajor(x1[:, sl]),
            "x2s": cmajor(x2bf[:, sl]),
            "vs": cmajor(vbf[:, sl]),
            "hs": np.ascontiguousarray(hbf[sl]),
            "decays": np.ascontiguousarray(c["_decay_full"][sl]),
            "db_rep": np.ascontiguousarray(db[sl][None, :]),
        }
        for nm in CONST_NAMES:
            m[nm] = c[nm]
        in_maps.append(m)
    return in_maps


def kernel(x1, x2, v, h, d_bias):
    from concourse import bass_utils

    nc = _get_nc()
    in_maps = make_in_maps(x1, x2, v, h, d_bias)
    res = bass_utils.run_bass_kernel_spmd(
        nc, in_maps, core_ids=list(range(NCORES)))
    # [dpc, B*L] per core -> [B, dpc, L], concat over cores
    outs = [r["out"].reshape(DPC, B, L).transpose(1, 0, 2)
            for r in res.results]
    out = np.concatenate(outs, axis=1)
    return np.ascontiguousarray(out.astype(np.float32))


if __name__ == "__main__":
    rng = np.random.default_rng(0)
    inputs = {
        "x1": rng.standard_normal((B, D, L)).astype(np.float32),
        "x2": rng.standard_normal((B, D, L)).astype(np.float32),
        "v": rng.standard_normal((B, D, L)).astype(np.float32),
        "h": (rng.standard_normal((D, L)) / math.sqrt(L) * 1e-5).astype(np.float32),
        "d_bias": rng.standard_normal(D).astype(np.float32),
    }
    out = kernel(**inputs)
    print(out.shape, out.dtype)
